# revision 17
# baseline (speedup 1.0000x reference)
"""Memory-efficient multi-head attention block on 8 TRN2 NeuronCores.

Computes (matching torch nn.Linear conventions, W is [out, in]):
    q, k, v = x@Wq.T, x@Wk.T, x@Wv.T          [B, S, H] -> heads [B, NH, S, HD]
    out     = softmax(q k^T / sqrt(HD)) v      per head
    y       = concat_heads(out) @ Wo.T + bo

Sharding: head-parallel tensor parallelism. Each of the 8 cores owns 2 of the
16 heads: Wq/Wk/Wv are sharded on their output dim, Wo on its input dim. Each
core computes a full-shape partial y (its heads' contribution through Wo);
host sums the 8 partials and adds the bias.

Layout trick: everything on device runs transpose-free.  The host feeds
x pre-transposed as xT[b] = x[b].T ([H, S]); then
  - qT/kT (per head [HD, S]) come out of matmuls directly (weights as lhsT),
  - v     (per head [S, HD]) uses xT tiles as lhsT,
  - scores^T [k, q] = kT_tile.T @ qT (contract over HD partitions),
  - attn-out^T [HD, q] = v_tile.T @ exp(scores^T) (contract over k partitions),
  - y tiles [tok, out] = attnT_tile.T @ WoT (contract over HD partitions).
Softmax skips the max-subtraction (scaled scores are ~N(0,1); exp is safe in
fp32) and builds the denominator with a DVE accumulation over k-tiles plus an
all-ones matmul that both finishes the sum across partitions and broadcasts
it; 1/norm is exp(-ln(norm)) on ScalarE (exp and ln share one ACT table set;
the dedicated Reciprocal op is disallowed/slow).

Matmuls run in float32r (TF32-like: full rate at free-dim >= 256, measured
rel. err ~1.5e-4 for a K=2048 contraction vs fp32's 4x slowdown).

All PSUM tiles are [128, 1024] (two banks).  Where two matmul accumulation
groups share one tile they are kept bank-disjoint (a `start=True` clear is
bank-granular, so two groups in one bank corrupt each other).  Query tiles are
processed in pairs so exp / row-sum accumulation run 1024 wide, amortizing the
per-instruction overhead of ScalarE/VectorE.
"""
import sys

sys.path.insert(0, "/opt/trn_rl_repo")

import numpy as np

import concourse.bass as bass  # noqa: F401  (engine registry import side effects)
import concourse.mybir as mybir
import concourse.tile as tile
from concourse import bacc
from concourse.bass_utils import run_bass_kernel_spmd

B, S, H, NH = 2, 2048, 2048, 16
HD = H // NH            # 128
NCORES = 8
HPC = NH // NCORES      # heads per core = 2
DLOC = HPC * HD         # 256 local head dims per core
SCALE = 1.0 / float(np.sqrt(HD))

F32 = mybir.dt.float32
F32R = mybir.dt.float32r
BF16 = mybir.dt.bfloat16
# dtype for the attention-phase matmul operands (qT/kT/v/exp-scores).
# float32r: TF32-like, serialized weight loads; bfloat16: hidden weight
# loads (separate LDWEIGHTS + fast-weight-load) but coarser rounding.
ATT_DT = F32R
EXP = mybir.ActivationFunctionType.Exp
LN = mybir.ActivationFunctionType.Ln

KS = H // 128           # 16 contraction subtiles for the projections
NTT = S // 512          # 4 token tiles of 512 (projection rhs width)
NKT = S // 128          # 16 key tiles of 128
NPR = S // 1024         # 2 query-pair tiles of 1024
NTT2 = S // 128         # 16 token tiles of 128 (output projection)
NOT2 = H // 1024        # 2 output tiles of 1024 (output projection)


_TABLES_PATCHED = False


def _patch_act_tables():
    """Make Exp and Ln resolve to the one table set that holds both.

    bacc's insert_act_table_loads picks the first act_info.json set
    containing each function, which puts Exp in `exp_and_others` and Ln in
    `natural_log` — every softmax-normalize then pays two ~1.3us
    ACT_TABLE_LOADs.  Hiding exp/ln from every set except
    `natural_log_exp_and_others` (indices preserved) makes the chooser load
    one set, once.
    """
    global _TABLES_PATCHED
    if _TABLES_PATCHED:
        return
    _TABLES_PATCHED = True
    orig = bacc.get_activation_tables

    def patched(arch):
        tabs = orig(arch)
        out = {}
        for name, fns in tabs.items():
            if name != "natural_log_exp_and_others":
                fns = fns - {EXP, LN}
            out[name] = fns
        return out

    bacc.get_activation_tables = patched


def _build():
    _patch_act_tables()
    nc = bacc.Bacc("TRN2", target_bir_lowering=False, debug=False, num_devices=NCORES)

    xT_d = nc.dram_tensor("xT", [B, H, S], F32R, kind="ExternalInput").ap()
    ones_d = nc.dram_tensor("ones", [128, 128], F32R, kind="ExternalInput").ap()
    wq_d = nc.dram_tensor("wq", [H, DLOC], F32R, kind="ExternalInput").ap()
    wk_d = nc.dram_tensor("wk", [H, DLOC], F32R, kind="ExternalInput").ap()
    wv_d = nc.dram_tensor("wv", [H, DLOC], F32R, kind="ExternalInput").ap()
    wo_d = nc.dram_tensor("wo", [DLOC, H], F32R, kind="ExternalInput").ap()
    y_d = nc.dram_tensor("y", [B, S, H], F32, kind="ExternalOutput").ap()

    with tile.TileContext(nc) as tc:
        with tc.tile_pool(name="sb", bufs=1) as sb, \
             tc.tile_pool(name="ps", bufs=1, space="PSUM") as ps:

            def p2(name):
                return ps.tile([128, 1024], F32, tag="p2", bufs=3, name=name)

            def p1(name):
                return ps.tile([128, 512], F32, tag="p1", bufs=2, name=name)

            ones = sb.tile([128, 128], F32R, tag="ones", bufs=1)

            wq_s = sb.tile([128, KS, DLOC], F32R, tag="wq", bufs=1)
            wk_s = sb.tile([128, KS, DLOC], F32R, tag="wk", bufs=1)
            wv_s = sb.tile([128, KS, DLOC], F32R, tag="wv", bufs=1)
            wo_s = sb.tile([128, HPC, H], F32R, tag="wo", bufs=1)
            for ks in range(KS):
                nc.gpsimd.dma_start(wq_s[:, ks], wq_d[ks * 128:(ks + 1) * 128, :])
                nc.gpsimd.dma_start(wk_s[:, ks], wk_d[ks * 128:(ks + 1) * 128, :])
                nc.gpsimd.dma_start(wv_s[:, ks], wv_d[ks * 128:(ks + 1) * 128, :])
            nc.gpsimd.dma_start(ones, ones_d)
            for h in range(HPC):
                nc.gpsimd.dma_start(wo_s[:, h], wo_d[h * 128:(h + 1) * 128, :])

            qTb = [None] * B
            kTb = [None] * B
            vb = [None] * B
            aoTb = [None] * B

            def proj(b):
                # q/k/v projections: one streaming pass over xT[b].
                qTb[b] = sb.tile([128, HPC, S], ATT_DT, tag="qTb", bufs=1, name=f"qTb{b}")
                kTb[b] = sb.tile([128, HPC, S], ATT_DT, tag="kTb", bufs=1, name=f"kTb{b}")
                vb[b] = sb.tile([128, NKT, DLOC], ATT_DT, tag="vb", bufs=1, name=f"vb{b}")
                aoTb[b] = sb.tile([128, HPC, S], F32R, tag="aoTb", bufs=1, name=f"aoTb{b}")
                for tt in range(NTT):
                    # two heads' q (or k) share one 2-bank psum tile, one bank
                    # per head; v gets one bank per 128-token subtile (only
                    # 256 of each bank's 512 lanes are used).
                    q_ps = p2(f"qps{b}{tt}")
                    k_ps = p2(f"kps{b}{tt}")
                    v_pair = p2(f"vps{b}{tt}")
                    v_one = [p1(f"vpo{b}{tt}{i}") for i in range(2)]
                    v_slots = [v_pair[:, 0:256], v_pair[:, 512:768],
                               v_one[0][:, 0:256], v_one[1][:, 0:256]]
                    for ks in range(KS):
                        xs = sb.tile([128, 512], F32R, tag="xs", bufs=6, name=f"xs{b}{tt}{ks}")
                        nc.sync.dma_start(
                            xs, xT_d[b, ks * 128:(ks + 1) * 128, tt * 512:(tt + 1) * 512])
                        st, sp = ks == 0, ks == KS - 1
                        for m in range(2):
                            nc.tensor.matmul(q_ps[:, m * 512:(m + 1) * 512],
                                             wq_s[:, ks, m * 128:(m + 1) * 128],
                                             xs, start=st, stop=sp)
                        for m in range(2):
                            nc.tensor.matmul(k_ps[:, m * 512:(m + 1) * 512],
                                             wk_s[:, ks, m * 128:(m + 1) * 128],
                                             xs, start=st, stop=sp)
                        for t4 in range(4):
                            nc.tensor.matmul(
                                v_slots[t4],
                                xs[:, t4 * 128:(t4 + 1) * 128], wv_s[:, ks],
                                start=st, stop=sp)
                    for m in range(2):
                        nc.vector.tensor_copy(qTb[b][:, m, tt * 512:(tt + 1) * 512],
                                              q_ps[:, m * 512:(m + 1) * 512])
                        nc.vector.tensor_copy(kTb[b][:, m, tt * 512:(tt + 1) * 512],
                                              k_ps[:, m * 512:(m + 1) * 512])
                    for t4 in range(4):
                        nc.vector.tensor_copy(vb[b][:, tt * 4 + t4, :], v_slots[t4])

            def attn_loop(b, h, pr):
                """kt loop for one 1024-wide query pair; returns tail closure.

                Emission is software-pipelined: scores(kt+1) precedes
                attnv(kt) so the (in-order) PE stream never waits on exp(kt)
                to issue the next scores.  The PSUM-freeing copy of the
                attn-out accumulator is eager; the normalize chain (ones-
                matmul -> ln -> exp -> mul), which trails the serial DVE
                row-sum chain, is returned for deferred emission so it never
                blocks later matmuls in the PE stream.
                """
                unn2 = p2(f"unn{b}{h}{pr}")
                # two interleaved row-sum chains (even/odd kt) halve the
                # serial DVE add recurrence; the ones-matmuls combine them.
                accA = sb.tile([128, 1024], F32R, tag="acc", bufs=4, name=f"accA{b}{h}{pr}")
                accB = sb.tile([128, 1024], F32R, tag="acc", bufs=4, name=f"accB{b}{h}{pr}")
                q0 = pr * 1024
                e2s = [None] * NKT

                def scores(kt):
                    s2 = p2(f"sps{b}{h}{pr}{kt}")
                    for i in range(2):
                        nc.tensor.matmul(
                            s2[:, i * 512:(i + 1) * 512],
                            kTb[b][:, h, kt * 128:(kt + 1) * 128],
                            qTb[b][:, h, q0 + i * 512:q0 + (i + 1) * 512],
                            start=True, stop=True)
                    e2 = sb.tile([128, 1024], ATT_DT, tag="e", bufs=4, name=f"e{b}{h}{pr}{kt}")
                    nc.scalar.activation(e2, s2, EXP, scale=SCALE)
                    e2s[kt] = e2

                def attnv(kt):
                    st, sp = kt == 0, kt == NKT - 1
                    e2 = e2s[kt]
                    for i in range(2):
                        nc.tensor.matmul(
                            unn2[:, i * 512:(i + 1) * 512],
                            vb[b][:, kt, h * 128:(h + 1) * 128],
                            e2[:, i * 512:(i + 1) * 512],
                            start=st, stop=sp)
                    tgt = accA if kt % 2 == 0 else accB
                    if kt < 2:
                        nc.vector.tensor_copy(tgt, e2)
                    else:
                        nc.vector.tensor_add(tgt, tgt, e2)

                scores(0)
                for kt in range(1, NKT):
                    scores(kt)
                    attnv(kt - 1)
                attnv(NKT - 1)
                # eager: free the two unn banks
                ub2 = sb.tile([128, 1024], F32, tag="ub", bufs=3, name=f"ub{b}{h}{pr}")
                nc.vector.tensor_copy(ub2, unn2)

                def tail():
                    for i in range(2):
                        nps = p1(f"nps{b}{h}{pr}{i}")
                        nc.tensor.matmul(nps, ones, accA[:, i * 512:(i + 1) * 512],
                                         start=True, stop=False)
                        nc.tensor.matmul(nps, ones, accB[:, i * 512:(i + 1) * 512],
                                         start=False, stop=True)
                        lnn = sb.tile([128, 512], F32, tag="rc", bufs=4, name=f"ln{b}{h}{pr}{i}")
                        nc.scalar.activation(lnn, nps, LN)
                        rc = sb.tile([128, 512], F32, tag="rc", bufs=4, name=f"rc{b}{h}{pr}{i}")
                        nc.scalar.activation(rc, lnn, EXP, scale=-1.0)
                        nc.vector.tensor_mul(aoTb[b][:, h, q0 + i * 512:q0 + (i + 1) * 512],
                                             ub2[:, i * 512:(i + 1) * 512], rc)
                return tail

            def outproj(b, t2s=range(NTT2)):
                for t2 in t2s:
                    for ot in range(4):
                        y1 = p1(f"yps{b}{t2}{ot}")
                        for h in range(HPC):
                            nc.tensor.matmul(
                                y1, aoTb[b][:, h, t2 * 128:(t2 + 1) * 128],
                                wo_s[:, h, ot * 512:(ot + 1) * 512],
                                start=(h == 0), stop=(h == HPC - 1))
                        ysb = sb.tile([128, 512], F32, tag="ysb", bufs=4, name=f"ysb{b}{t2}{ot}")
                        nc.vector.tensor_copy(ysb, y1)
                        nc.sync.dma_start(
                            y_d[b, t2 * 128:(t2 + 1) * 128, ot * 512:(ot + 1) * 512], ysb)

            # Normalize tails are deferred by at least one full kt-loop of
            # PE work so their acc-chain dependency is settled by the time
            # the in-order PE stream reaches them.
            proj(0)
            t000 = attn_loop(0, 0, 0)
            t001 = attn_loop(0, 0, 1)
            t000()
            t010 = attn_loop(0, 1, 0)
            t001()
            t011 = attn_loop(0, 1, 1)
            t010()
            proj(1)
            t011()
            t100 = attn_loop(1, 0, 0)
            t101 = attn_loop(1, 0, 1)
            t100()
            outproj(0)   # PE-heavy; fills batch-1 attention's ACT-bound gaps
            t101()
            t110 = attn_loop(1, 1, 0)
            t111 = attn_loop(1, 1, 1)
            t110()
            # token tiles 0..7 only need the pr=0 tails; emit them before the
            # final pr=1 tail so they overlap the last attention loop's
            # ACT/DVE-bound stretch.
            outproj(1, range(NTT2 // 2))
            t111()
            outproj(1, range(NTT2 // 2, NTT2))

    nc.compile()
    return nc


_NC = None


def _get_nc():
    global _NC
    if _NC is None:
        _NC = _build()
    return _NC


def kernel(x, Wq, Wk, Wv, Wo, bo):
    x = np.asarray(x, dtype=np.float32)
    Wq = np.asarray(Wq, dtype=np.float32)
    Wk = np.asarray(Wk, dtype=np.float32)
    Wv = np.asarray(Wv, dtype=np.float32)
    Wo = np.asarray(Wo, dtype=np.float32)
    bo = np.asarray(bo, dtype=np.float32)

    nc = _get_nc()
    xT = np.ascontiguousarray(x.transpose(0, 2, 1))
    in_maps = []
    for c in range(NCORES):
        sl = slice(c * DLOC, (c + 1) * DLOC)
        in_maps.append({
            "xT": xT,
            "ones": np.ones((128, 128), dtype=np.float32),
            "wq": np.ascontiguousarray(Wq[sl, :].T),
            "wk": np.ascontiguousarray(Wk[sl, :].T),
            "wv": np.ascontiguousarray(Wv[sl, :].T),
            "wo": np.ascontiguousarray(Wo[:, sl].T),
        })
    res = run_bass_kernel_spmd(nc, in_maps, list(range(NCORES)))
    y = np.zeros((B, S, H), dtype=np.float32)
    for c in range(NCORES):
        y += np.asarray(res.results[c]["y"])
    y += bo
    return y
